# revision 6
# baseline (speedup 1.0000x reference)
"""Trainium2 Bass kernel for a 3x3 stride-1 pad-1 Conv2d (NCHW).

Problem (hardcoded): x (16, 128, 128, 128) f32, K (3, 3, 128, 256) f32.
The reference reinterprets K's flat buffer as (Cin, kh, kw, Cout) and only
writes output rows/cols 0..124 (the rest of the 128x128 output stays zero).

Strategy: data-parallel over batch — 2 images per NeuronCore on 8 cores.
Per image, the conv is 32 output tiles of 4 rows x 125 valid cols, each
accumulated over the 9 taps into one PSUM bank per Cout half (contraction
over Cin=128 partitions): 1152 matmuls/core at N=500, streaming at the
PE roofline (~213 ns/MM measured, 1 col/cycle @ 2.4 GHz).

Operands are bf16 (rel_l2 ~2.4e-3 vs the 2e-2 gate): fp32r's weight path
adds a fixed ~20 ns/MM that bf16 avoids, and an fp32r ISA rule forbids
the odd innermost count N=500 needs. Accumulation stays fp32 in PSUM.
The last row-tile of each (image, half) computes only output row 124
(N=125) instead of a 4-row tile with three garbage rows.

The activation plane is zero-padded to 130 rows x 128 cols (cols -1..126;
original col 127 only feeds invalid outputs) and streams in row-band
chunks, smallest first, so the first matmul fires as soon as the runtime
startup (~8.5 us) allows. Weights load per-tap so no matmul stalls on a
bulk transfer. Dummy matmuls on a zeroed tile warm the PE's HAM clock
gate (1.2 -> 2.4 GHz) inside the DMA shadow. Stores ship full 4x128-row
tiles (2KB contiguous per channel); cols/rows >= 125 carry garbage that
the host zeroes after the gather.
"""

import ml_dtypes
import numpy as np

import concourse.bacc as bacc
import concourse.mybir as mybir
import concourse.tile as tile
from concourse.bass_utils import run_bass_kernel_spmd

N_CORES = 8
B, CIN, H, W = 16, 128, 128, 128
COUT = 256
BPC = B // N_CORES
HP = H + 2
WPAD = 128  # padded cols -1..126 (col 127 only feeds invalid outputs)
VALID = 125
ROWS_PER_TILE = 4
F32 = mybir.dt.float32
BF16 = mybir.dt.bfloat16

CHUNKS = [(0, 2), (2, 3), (5, 4), (9, 4), (13, 4), (17, 4), (21, 4), (25, 3), (28, 3), (31, 1)]
CHUNK_MAX_ROWS = 4 * 4 + 2

_NC_CACHE = {}


def _build_nc(reps=1):
    nc = bacc.Bacc()
    x_in = nc.dram_tensor("x", [BPC, CIN, HP, WPAD], BF16, kind="ExternalInput")
    w_in = nc.dram_tensor("w", [CIN, 9 * COUT], BF16, kind="ExternalInput")
    out_t = nc.dram_tensor("out", [BPC, COUT, H * W], F32, kind="ExternalOutput")

    with tile.TileContext(nc) as tc:
        with (
            tc.tile_pool(name="wpool", bufs=1) as wpool,
            tc.tile_pool(name="dpool", bufs=1) as dpool,
            tc.tile_pool(name="xpool", bufs=4) as xpool,
            tc.tile_pool(name="opool", bufs=6) as opool,
            tc.tile_pool(name="pspool", bufs=7, space="PSUM") as pspool,
            tc.tile_pool(name="psdummy", bufs=1, space="PSUM") as psdummy,
        ):
            w_sb = wpool.tile([CIN, 9 * COUT], BF16)
            # Tap-0 weights land before the first x chunk; the remaining
            # taps stream per-tap behind chunk0, each arriving just ahead
            # of the matmul that needs it.
            nc.sync.dma_start(out=w_sb[:, 0:256], in_=w_in[:, 0:256])

            # PE pre-warm: the HAM clock gate keeps the PE at 1.2 GHz until
            # ~3.4us of sustained matmul activity. The first real matmul
            # can't fire until w+chunk0 land (~11us); fill that DMA shadow
            # with dummy matmuls on never-written SBUF (no deps, results
            # discarded) so the real stream starts at 2.4 GHz.
            d_x = dpool.tile([CIN, 512], BF16)
            d_ps = psdummy.tile([128, 512], F32)
            nc.vector.memset(d_x[:], 0)
            for j in range(14):
                nc.tensor.matmul(
                    d_ps[:], d_x[:, 0:128], d_x[:], start=True, stop=True
                )

            first = True
            for b in [b for _ in range(reps) for b in range(BPC)]:
                for rb0, ntiles in CHUNKS:
                    r0 = 4 * rb0
                    nrows = 4 * ntiles + 2
                    xc = xpool.tile([CIN, CHUNK_MAX_ROWS, WPAD], BF16)
                    nc.sync.dma_start(
                        out=xc[:, 0:nrows, :],
                        in_=x_in[b, :, r0 : r0 + nrows, :],
                    )
                    if first:
                        for t in range(1, 9):
                            nc.sync.dma_start(
                                out=w_sb[:, t * 256 : (t + 1) * 256],
                                in_=w_in[:, t * 256 : (t + 1) * 256],
                            )
                        first = False
                    for lrb in range(ntiles):
                        rb = rb0 + lrb
                        r = 4 * rb
                        lr = 4 * lrb
                        # Tile 31 only has one valid output row (124); rows
                        # 125..127 would be garbage — compute just row 124
                        # (N=125 per tap) instead of a full 4-row tile.
                        nrow = 1 if rb == 31 else ROWS_PER_TILE
                        for c2 in range(2):
                            ps = pspool.tile(
                                [128, ROWS_PER_TILE, VALID], F32, tag="ps"
                            )
                            for i, t in enumerate(range(9)):
                                kh, kw = divmod(t, 3)
                                c0 = t * COUT + c2 * 128
                                nc.tensor.matmul(
                                    ps[:, 0:nrow, :],
                                    w_sb[:, c0 : c0 + 128],
                                    xc[
                                        :,
                                        lr + kh : lr + kh + nrow,
                                        kw : kw + VALID,
                                    ],
                                    start=(i == 0),
                                    stop=(i == 8),
                                )
                            ob = opool.tile(
                                [128, ROWS_PER_TILE, W], F32, tag="ob"
                            )
                            nc.vector.tensor_copy(
                                out=ob[:, 0:nrow, 0:VALID], in_=ps[:, 0:nrow, :]
                            )
                            nc.sync.dma_start(
                                out=out_t[
                                    b,
                                    c2 * 128 : (c2 + 1) * 128,
                                    r * W : r * W + nrow * W,
                                ],
                                in_=ob[:, 0:nrow, :],
                            )
    nc.finalize()
    return nc


def _get_nc(reps=1):
    if reps not in _NC_CACHE:
        _NC_CACHE[reps] = _build_nc(reps)
    return _NC_CACHE[reps]


def _run(x, K, trace=False, reps=1):
    x_pad = np.zeros((B, CIN, HP, WPAD), dtype=ml_dtypes.bfloat16)
    x_pad[:, :, 1 : H + 1, 1:WPAD] = np.asarray(x, dtype=np.float32)[
        :, :, :, 0 : WPAD - 1
    ].astype(ml_dtypes.bfloat16)
    w_host = (
        np.asarray(K, dtype=np.float32)
        .reshape(CIN, 9 * COUT)
        .astype(ml_dtypes.bfloat16)
    )
    in_maps = [
        {"x": x_pad[i * BPC : (i + 1) * BPC], "w": w_host} for i in range(N_CORES)
    ]
    res = run_bass_kernel_spmd(
        _get_nc(reps), in_maps, list(range(N_CORES)), trace=trace
    )
    out = np.concatenate(
        [res.results[i]["out"].reshape(BPC, COUT, H, W) for i in range(N_CORES)],
        axis=0,
    )
    out[:, :, VALID:, :] = 0
    out[:, :, :, VALID:] = 0
    return out, res


def kernel(x, K):
    out, _ = _run(x, K, trace=False)
    return out
